# revision 5
# baseline (speedup 1.0000x reference)
"""GAU attention (gated attention unit) Trainium2 Bass kernel.

Reference computation (B=2, S=2048, D=1024, H=16, DH=64):
    q = (hs @ Wq + bq), k = (hs @ Wk + bk), v = (hs @ Wv + bv)   per-head [B,S,H,DH]
    scores = q k^T / sqrt(DH);  probs = softmax(scores, axis=k)
    gating = sigmoid(gf * mean_d(hs) + gb)          # [B, S] per (batch, query)
    ctx = (probs * gating) @ v;  out = ctx @ Wo + bo

Sharding: 8 cores = 2 batches x 4 head-groups (4 heads each).  Each core
computes out^T partial [D, S] for its (batch, head-group); host sums the 4
partials per batch, applies the per-(batch,query) gating scalar (it commutes
to the output), transposes, and adds bo.

Bias algebra (exact, done on host so the device never sees bk/bv):
  - bk: scores_q += q . bk is constant over the softmax (key) axis -> cancels.
  - bv: ctx = sum_j p_j (v_j + bv) = sum_j p_j v_j + bv (probs sum to 1 per
    head), so out += Wo^T bv, applied on the host as a gating outer product.

Per-core dataflow (all matmuls bf16 with fp32 PSUM accumulation):
  - hs^T [D,S] staged bf16 (host transposes + casts).
  - Q^T,K^T [256,S]: lhsT=W tiles (stationary), rhs=hs^T.  Layout: pair p of
    heads stacked on partitions (head A dh on 0:64, head B on 64:128).
  - K-proj/V-proj/Q-proj are emitted just-in-time inside the first q-chunk's
    attention loop so the exp (ACT) stream starts early.
  - scores^T [k,q] per (pr, kt): two row-packed (tile_position (0,0)/(64,0))
    K=64 matmuls -> exp -> E^T bf16.  Exps run on ACT (table exp, scale=ln2
    over log2-space scores) except a few per chunk offloaded to DVE via a
    Schraudolph bit-trick: i16 = round(y*128 + (127-C)*128) reinterpreted as
    bf16 gives 2^y to ~2%; softmax-consistent so it perturbs ctx ~0.1%.
  - softmax denom: E^T ktiles folded into 4 partial sums per pr (kt%4
    chains) -- 3 chains on DVE, 1 on GPSIMD -- then col-packed ones-matmuls
    broadcast both heads' denominators into one [128,GQ] PSUM tile
    (accumulating the 4 partials), one reciprocal, one multiply.
  - AV: col-packed (tile_position (0,0)/(0,64)) matmuls, V stationary,
    E^T streaming -> ctx^T accumulated over ktiles in a single PSUM bank per
    pr (disjoint partition ranges; skip_group_check).
  - O-proj lhsT=Wo, rhs=ctx^T (bf16, already 1/denom-scaled) -> out^T.
  - Prologue: critical DMA (wk/wq m-halves, hs chunk0 d-tiles) spread over 4
    DGE rings (sync/scalar/gpsimd/vector) in consumption order; PE warm-up
    matmuls bridge the DMA window so projections start at full clock.
  - Tail: the last chunk's denominator/O-proj pipeline is split into q-halves
    of 256 and the output stores fan out over 3 rings.
"""

import sys

for _p in ("/opt/trn_rl_repo", "/root/.axon_site/_ro/trn_rl_repo"):
    if _p not in sys.path:
        sys.path.append(_p)

from contextlib import ExitStack

import ml_dtypes
import numpy as np

import concourse.bass as bass
import concourse.mybir as mybir
import concourse.tile as tile
from concourse import bacc
from concourse.bass_utils import run_bass_kernel_spmd

BF16 = mybir.dt.bfloat16
F32 = mybir.dt.float32
I16 = mybir.dt.int16
AF = mybir.ActivationFunctionType
OP = mybir.AluOpType

B, S, D, H = 2, 2048, 1024, 16
DH = 64
LN2 = float(np.log(2.0))
LOG2E = float(np.log2(np.e))
HPC = 4  # heads per core
GD = HPC * DH  # 256 (head-group width)
NCORES = 8
NDT = D // 128  # 8 contraction tiles over D

# Schraudolph 2^y for bf16: i16 = round(y*128 + (127 - C)*128), bits = bf16.
# C = 0.0430 is the mean-error-minimizing shift; +0.5 centers a truncating
# float->int conversion (harmless if the hw rounds: then it's C-1/256).
SCHRA_SCALE = 128.0
SCHRA_BIAS = (127.0 - 0.0430) * 128.0 + 0.5
# (qc>0 only) exps offloaded from ACT to DVE: pr==1 and kt in this set
DVE_EXP_KT = (3, 7, 11)


def _build(ctx: ExitStack, tc: "tile.TileContext", io: dict, s: int):
    nc = tc.nc
    GQ = min(512, s)
    NQC = s // GQ  # q chunks
    NKT = s // 128  # k tiles

    hsT, wq, wk, wv, wo = io["hsT"], io["wq"], io["wk"], io["wv"], io["wo"]
    bq, outT = io["bq"], io["outT"]

    consts = ctx.enter_context(tc.tile_pool(name="consts", bufs=1))
    sb = ctx.enter_context(tc.tile_pool(name="sb", bufs=1))
    # et tiles must stay live for NPAR=4 k-tiles (first fold of each chain
    # consumes et(kt-4)), 2 tiles per kt -> 16 buffers
    etp = ctx.enter_context(tc.tile_pool(name="etp", bufs=16))
    ksp = ctx.enter_context(tc.tile_pool(name="ksp", bufs=2))
    outp = ctx.enter_context(tc.tile_pool(name="outp", bufs=8))
    # PSUM budget: 2x2 (scores, 2-bank tiles) + 2 (ctx, one bank per pr via
    # disjoint-partition accumulation groups) + 2 (vproj/denom/o-proj) = 8
    ps_mm = ctx.enter_context(tc.tile_pool(name="ps_mm", bufs=2, space="PSUM"))
    ps_ctx = ctx.enter_context(tc.tile_pool(name="ps_ctx", bufs=2, space="PSUM"))
    ps_o = ctx.enter_context(tc.tile_pool(name="ps_o", bufs=2, space="PSUM"))

    # ---- constants ----
    ones128 = consts.tile([128, 128], BF16, tag="ones128", name="ones128")
    nc.vector.memset(ones128[:], 1.0)

    bq_sb = consts.tile([128, 2], F32, tag="bq", name="bq")
    # explicit zero bias for Exp, written by DVE so the wait merges with the
    # DVE wait the exps already carry
    zbias = consts.tile([128, 1], F32, tag="zbias", name="zbias")
    nc.vector.memset(zbias[:], 0.0)
    # dummy exp as the very first ACT instruction: pulls the ~1.3us
    # ACT_TABLE_LOAD into the DMA-wait window
    warm = consts.tile([1, 1], F32, tag="warm", name="warm")
    nc.scalar.activation(warm[:], zbias[0:1, 0:1], AF.Exp, bias=zbias[0:1, 0:1], scale=1.0)

    # ---- weights + hs^T staged.  wk/wq live m-half-major [128,(m,d,128)] so
    # each m-half is one contiguous piece and kproj(m,0) starts on a 128KB
    # landing; hs^T as [128,(chunk,d,GQ)] with chunk0 sent per-d-tile. ----
    wk_all = consts.tile([128, NDT * GD], BF16, tag="wk", name="wk")
    wq_all = consts.tile([128, NDT * GD], BF16, tag="wq", name="wq")
    wv_all = consts.tile([128, NDT * GD], BF16, tag="wv", name="wv")
    CW = NDT * GQ  # 4096 columns per hs chunk block
    hsT_all = sb.tile([128, NDT * s], BF16, tag="hsT", name="hsT")

    def wslice(wall, m, d):  # [128,128] stationary tile for (m-half, d-tile)
        off = (m * NDT + d) * 128
        return wall[:, off : off + 128]

    wv_sb = [wv_all[:, d * GD : (d + 1) * GD] for d in range(NDT)]

    def hsq(d, qc):  # [128, GQ] tile of hs^T for (d-tile, q-chunk)
        off = qc * CW + d * GQ
        return hsT_all[:, off : off + GQ]

    def hsv(d, kt):  # [128, 128] tile of hs^T for (d-tile, k-tile)
        c, r = divmod(kt, 4)
        off = c * CW + d * GQ + r * 128
        return hsT_all[:, off : off + 128]

    # ---- DMA schedule: 3 DGE rings (sync/scalar/gpsimd), consumption order.
    # sync:   bq, wk-m0 (2 pieces), wq-m0, hs c1..c3, wo
    # scalar: hs-c0 odd d-tiles, wk-m1, wq-m1 (free again before the exps)
    # gpsimd: hs-c0 even d-tiles, wv
    HWM = NDT * 128  # 1024 cols per m-half of a weight tensor
    nc.sync.dma_start(bq_sb[:], bq.rearrange("(m p) -> p m", p=128))
    for dh in range(2):  # wk m0 in two 128KB pieces (d0-3, d4-7)
        cs = slice(dh * HWM // 2, (dh + 1) * HWM // 2)
        nc.sync.dma_start(wk_all[:, cs], wk[:, cs])
    for d in range(NDT):  # hs chunk0 per d-tile, alternating idle rings
        eng = nc.gpsimd if d % 2 == 0 else nc.scalar
        eng.dma_start(hsT_all[:, d * GQ : (d + 1) * GQ], hsT[:, d * GQ : (d + 1) * GQ])
    nc.sync.dma_start(wq_all[:, 0:HWM], wq[:, 0:HWM])
    nc.scalar.dma_start(wk_all[:, HWM : 2 * HWM], wk[:, HWM : 2 * HWM])
    nc.scalar.dma_start(wq_all[:, HWM : 2 * HWM], wq[:, HWM : 2 * HWM])
    nc.gpsimd.dma_start(wv_all[:], wv[:, :])
    for c in range(1, NQC):
        nc.sync.dma_start(hsT_all[:, c * CW : (c + 1) * CW], hsT[:, c * CW : (c + 1) * CW])
    wo_sb = [consts.tile([128, D], BF16, tag=f"wo{p}", name=f"wo{p}") for p in range(2)]
    nc.sync.dma_start(wo_sb[0][:], wo[0:128, :])
    nc.sync.dma_start(wo_sb[1][:], wo[128:256, :])

    qT_sb = [sb.tile([128, s], BF16, tag=f"qT{m}", name=f"qT{m}") for m in range(2)]
    kT_sb = [sb.tile([128, s], BF16, tag=f"kT{m}", name=f"kT{m}") for m in range(2)]
    v_sb = [sb.tile([128, GD], BF16, tag=f"v{st}", name=f"v{st}") for st in range(NKT)]

    def kproj(m, c):
        cc = slice(c * GQ, (c + 1) * GQ)
        p = ps_o.tile([128, GQ], F32, tag="po", name=f"kp{m}{c}")
        for d in range(NDT):
            nc.tensor.matmul(
                p[:], lhsT=wslice(wk_all, m, d), rhs=hsq(d, c),
                start=(d == 0), stop=(d == NDT - 1),
            )
        nc.vector.tensor_copy(kT_sb[m][:, cc], p[:])

    def qproj(m, qc):
        cc = slice(qc * GQ, (qc + 1) * GQ)
        p = ps_o.tile([128, GQ], F32, tag="po", name=f"qp{m}{qc}")
        for d in range(NDT):
            nc.tensor.matmul(
                p[:], lhsT=wslice(wq_all, m, d), rhs=hsq(d, qc),
                start=(d == 0), stop=(d == NDT - 1),
            )
        nc.vector.tensor_scalar_add(qT_sb[m][:, cc], p[:], bq_sb[:, m : m + 1])

    qchains = {}

    def qproj_half(m, qc, half):
        if half == 0:
            qchains[(m, qc)] = ps_o.tile([128, GQ], F32, tag="po", name=f"qph{m}{qc}")
        p = qchains[(m, qc)]
        for d in range(half * 4, half * 4 + 4):
            nc.tensor.matmul(
                p[:], lhsT=wslice(wq_all, m, d), rhs=hsq(d, qc),
                start=(d == 0), stop=(d == NDT - 1),
            )
        if half == 1:
            cc = slice(qc * GQ, (qc + 1) * GQ)
            nc.vector.tensor_scalar_add(qT_sb[m][:, cc], p[:], bq_sb[:, m : m + 1])

    def vproj(kt):
        vp = ps_o.tile([128, GD], F32, tag="po", name=f"vp{kt}")
        for d in range(NDT):
            nc.tensor.matmul(
                vp[:], lhsT=hsv(d, kt), rhs=wv_sb[d][:],
                start=(d == 0), stop=(d == NDT - 1),
            )
        nc.vector.tensor_copy(v_sb[kt][:], vp[:])

    # ---- PE warm-up: ~4.3us of dependency-free matmuls so the HAM clock
    # gate opens before the real (DMA-gated) projections run ----
    warm_ps = ps_o.tile([128, 128], F32, tag="po", name="warm_ps")
    NWARM = 40
    for i in range(NWARM):
        nc.tensor.matmul(
            warm_ps[:], lhsT=ones128[:], rhs=ones128[:],
            start=(i == 0), stop=(i == NWARM - 1),
        )

    # ---- prologue: just the pr0 projections; pr1's come as slot-0 filler
    # so the first exp fires after only two projection chains ----
    kproj(0, 0)
    qproj(0, 0)

    def oproj_unit(mt, ctx_sc, cs, copy_eng, dma_eng):
        ms = slice(mt * 128, (mt + 1) * 128)
        o_ps = ps_o.tile([128, GQ], F32, tag="po", name="po")
        for pr in range(2):
            nc.tensor.matmul(
                o_ps[:], lhsT=wo_sb[pr][:, ms], rhs=ctx_sc[pr][:],
                start=(pr == 0), stop=(pr == 1),
            )
        ost = outp.tile([128, GQ], BF16, tag="ost", name="ost")
        copy_eng(ost[:], o_ps[:])
        dma_eng.dma_start(outT[ms, cs], ost[:])

    # ---- per q-chunk attention, both head-pair groups interleaved per kt;
    # O-proj of chunk qc-1 trickles into qc's slots as PE filler ----
    NPAR = 4  # denominator partial-sum chains per (qc, pr), keyed by kt%4
    oproj_pending: list = []
    for qc in range(NQC):
        cs = slice(qc * GQ, (qc + 1) * GQ)
        ctx_ps = [ps_ctx.tile([128, GQ], F32, tag="ctx", name=f"ctx{pr}") for pr in range(2)]
        # per (pr, kt%4) partial exp-sums; chains 0-2 fold on DVE, 3 on GPSIMD
        ks = [[None] * NPAR, [None] * NPAR]
        ets = [[None] * NKT, [None] * NKT]
        for kt in range(NKT + 1):
            sps = [None, None]
            for pr in range(2):
                if kt < NKT:
                    ks_ = slice(kt * 128, (kt + 1) * 128)
                    sp = ps_mm.tile([128, 2 * GQ], F32, tag="smm", name="smm")
                    sps[pr] = sp
                    nc.tensor.matmul(
                        sp[:, 0:GQ], lhsT=kT_sb[pr][0:64, ks_], rhs=qT_sb[pr][0:64, cs],
                        tile_position=(0, 0), start=True, stop=True,
                    )
                    nc.tensor.matmul(
                        sp[:, GQ : 2 * GQ], lhsT=kT_sb[pr][64:128, ks_], rhs=qT_sb[pr][64:128, cs],
                        tile_position=(64, 0), start=True, stop=True,
                    )
                if pr == 0 and kt < NKT:
                    # just-in-time projections keep PE fed while ACT drains exps
                    if qc == 0:
                        if kt == 0:
                            kproj(1, 0)
                            qproj(1, 0)
                            vproj(0)
                        if kt + 1 < NKT:
                            vproj(kt + 1)
                        if kt in (1, 5, 9):
                            kproj(0, kt // 4 + 1)
                        elif kt in (2, 6, 10):
                            kproj(1, kt // 4 + 1)
                        elif kt == 12:
                            qproj(0, 1)
                        elif kt == 13:
                            qproj(1, 1)
                    else:
                        if qc < NQC - 1:
                            if kt in (5, 6):
                                qproj_half(0, qc + 1, kt - 5)
                            elif kt in (7, 8):
                                qproj_half(1, qc + 1, kt - 7)
                        if oproj_pending and kt >= 1 and (kt < 5 or kt > 8):
                            oproj_pending.pop(0)()
                if kt > 0:
                    pv = kt - 1
                    et = ets[pr][pv]
                    nc.tensor.matmul(
                        ctx_ps[pr][0:64, :], lhsT=v_sb[pv][:, pr * 128 : pr * 128 + 64],
                        rhs=et[:, 0:GQ], tile_position=(0, 0),
                        start=(pv == 0), stop=(pv == NKT - 1),
                        skip_group_check=True,
                    )
                    nc.tensor.matmul(
                        ctx_ps[pr][64:128, :], lhsT=v_sb[pv][:, pr * 128 + 64 : pr * 128 + 128],
                        rhs=et[:, GQ : 2 * GQ], tile_position=(0, 64),
                        start=(pv == 0), stop=(pv == NKT - 1),
                        skip_group_check=True,
                    )
            for pr in range(2):
                if kt < NKT:
                    et = etp.tile([128, 2 * GQ], BF16, tag="et", name="et")
                    ets[pr][kt] = et
                    if qc > 0 and pr == 1 and kt in DVE_EXP_KT:
                        # Schraudolph on DVE: scores arrive in log2 space
                        # (log2e/8 folded into Wq); one fused mul+add into an
                        # int16 view of the bf16 tile computes 2^y bitwise
                        nc.vector.tensor_scalar(
                            et[:].bitcast(I16), sps[pr][:],
                            SCHRA_SCALE, SCHRA_BIAS, op0=OP.mult, op1=OP.add,
                        )
                    else:
                        # exp(ln2*y) = 2^y on ACT
                        nc.scalar.activation(et[:], sps[pr][:], AF.Exp, bias=zbias[:, 0:1], scale=LN2)
                    par = kt % NPAR
                    if kt >= NPAR:
                        fold_eng = nc.gpsimd if par == 3 else nc.vector
                        if kt < 2 * NPAR:
                            # first fold is out-of-place (no seed copy needed)
                            kst = ksp.tile([128, 2 * GQ], BF16, tag=f"ks{pr}{par}", name=f"ks{pr}{par}")
                            ks[pr][par] = kst
                            fold_eng.tensor_tensor(kst[:], ets[pr][par][:], et[:], op=OP.add)
                        else:
                            kst = ks[pr][par]
                            fold_eng.tensor_tensor(kst[:], kst[:], et[:], op=OP.add)

        # softmax denominators: col-packed ones-matmul pairs broadcast both
        # heads into one bank, accumulating the 4 partial chains
        def denom_ctx(pr, qh=None):
            # qh=None: full GQ; else a 256-wide half for the tail pipeline
            qs = slice(0, GQ) if qh is None else slice(qh * (GQ // 2), (qh + 1) * (GQ // 2))
            w = qs.stop - qs.start
            db = ps_o.tile([128, w], F32, tag="po", name=f"db{pr}{qh}")
            for par in range(NPAR):
                for hh in range(2):
                    nc.tensor.matmul(
                        db[hh * 64 : (hh + 1) * 64, :],
                        lhsT=ones128[:, hh * 64 : (hh + 1) * 64],
                        rhs=ks[pr][par][:, hh * GQ + qs.start : hh * GQ + qs.stop],
                        tile_position=(0, hh * 64),
                        start=(par == 0), stop=(par == NPAR - 1),
                        skip_group_check=True,
                    )
            r = ksp.tile([128, w], F32, tag=f"r{pr}{qh}", name=f"r{pr}{qh}")
            nc.vector.reciprocal_approx_fast(r[:], db[:])
            sc = sb.tile([128, w], BF16, tag=f"ctxs{pr}_{qc % 2}_{qh}", name=f"ctxs{pr}_{qc % 2}_{qh}")
            nc.vector.tensor_tensor(sc[:], ctx_ps[pr][:, qs], r[:], op=OP.mult)
            return sc

        if qc < NQC - 1:
            # deferred into the next chunk's slots as PE filler
            ctx_sc = [denom_ctx(0), denom_ctx(1)]
            oproj_pending = [
                (lambda mt=mt, sc2=list(ctx_sc), c=cs: oproj_unit(
                    mt, sc2, c, nc.vector.tensor_copy, nc.sync))
                for mt in range(D // 128)
            ]
        else:
            # tail: pipeline in q-halves of 256 so the denominator/O-proj
            # drain overlaps; stores fan out over 3 DGE rings
            GH = GQ // 2
            for qh in range(2):
                sc_h = [denom_ctx(0, qh), denom_ctx(1, qh)]
                ch = slice(qc * GQ + qh * GH, qc * GQ + (qh + 1) * GH)
                for mt in range(D // 128):
                    ms = slice(mt * 128, (mt + 1) * 128)
                    o_ps = ps_o.tile([128, GH], F32, tag="po", name="po")
                    for pr in range(2):
                        nc.tensor.matmul(o_ps[:], lhsT=wo_sb[pr][:, ms], rhs=sc_h[pr][:],
                                         start=(pr == 0), stop=(pr == 1))
                    ost = outp.tile([128, GH], BF16, tag="ost", name="ost")
                    copy_eng = nc.vector.tensor_copy if mt % 2 == 0 else nc.scalar.copy
                    copy_eng(ost[:], o_ps[:])
                    dma_eng = (nc.sync, nc.scalar, nc.gpsimd)[mt % 3]
                    dma_eng.dma_start(outT[ms, ch], ost[:])


def build_gau_nc(s: int = S, debug: bool = False):
    nc = bacc.Bacc("TRN2", target_bir_lowering=False, debug=debug, num_devices=NCORES)
    io = {
        "hsT": nc.dram_tensor("hsT", [128, (D // 128) * s], BF16, kind="ExternalInput").ap(),
        "wq": nc.dram_tensor("wq", [128, (D // 128) * GD], BF16, kind="ExternalInput").ap(),
        "wk": nc.dram_tensor("wk", [128, (D // 128) * GD], BF16, kind="ExternalInput").ap(),
        "wv": nc.dram_tensor("wv", [128, (D // 128) * GD], BF16, kind="ExternalInput").ap(),
        "wo": nc.dram_tensor("wo", [GD, D], BF16, kind="ExternalInput").ap(),
        "bq": nc.dram_tensor("bq", [GD], F32, kind="ExternalInput").ap(),
        "outT": nc.dram_tensor("outT", [D, s], BF16, kind="ExternalOutput").ap(),
    }
    with tile.TileContext(nc) as tc:
        with ExitStack() as ctx:
            _build(ctx, tc, io, s)
    nc.compile()
    return nc


def make_in_maps(hidden_states, Wq, bq, Wk, bk, Wv, bv, Wo, gating_factor, gating_bias):
    """Shard full inputs into 8 per-core input maps (host-side prep)."""
    bf = ml_dtypes.bfloat16
    f32 = np.float32
    hs = np.asarray(hidden_states, f32)
    Wq, Wk, Wv, Wo = (np.asarray(a, f32) for a in (Wq, Wk, Wv, Wo))
    bq = np.asarray(bq, f32)

    # hs^T pre-shuffled to [128, (chunk, d, 512)] so the device DMA is a
    # plain contiguous 2D block; wq/wk to m-half-major [128, (m, d, 128)];
    # wv to [128, (d, GD)]
    def shuf_hs(a):  # a: [S, D]
        return np.ascontiguousarray(
            a.reshape(S // 512, 512, D // 128, 128).transpose(3, 0, 2, 1).reshape(128, -1)
        ).astype(bf)

    def shuf_w_m(w):  # w: [D, GD] -> [128, (m, d, 128)]
        return np.ascontiguousarray(
            w.reshape(D // 128, 128, 2, 128).transpose(1, 2, 0, 3).reshape(128, -1)
        ).astype(bf)

    def shuf_w(w):  # w: [D, GD] -> [128, (d, GD)]
        return np.ascontiguousarray(
            w.reshape(D // 128, 128, GD).transpose(1, 0, 2).reshape(128, -1)
        ).astype(bf)

    hsT_b = [shuf_hs(hs[b]) for b in range(B)]
    in_maps = []
    for c in range(NCORES):
        b, g = divmod(c, NCORES // B)
        cols = slice(g * GD, (g + 1) * GD)
        in_maps.append(
            {
                "hsT": hsT_b[b],
                "wq": shuf_w_m(Wq[:, cols] * np.float32(LOG2E / 8.0)),
                "wk": shuf_w_m(Wk[:, cols]),
                "wv": shuf_w(Wv[:, cols]),
                "wo": np.ascontiguousarray(Wo[cols, :]).astype(bf),
                "bq": np.ascontiguousarray(bq[cols] * np.float32(LOG2E / 8.0)),
            }
        )
    return in_maps


_NC_CACHE: dict = {}


def _get_nc(s: int = S):
    if s not in _NC_CACHE:
        _NC_CACHE[s] = build_gau_nc(s)
    return _NC_CACHE[s]


def run_gau(in_maps, **kwargs):
    nc = _get_nc(S)
    return run_bass_kernel_spmd(nc, in_maps, core_ids=list(range(NCORES)), **kwargs)


def assemble_output(results, bo, bv, Wo, gating):
    """Sum per-batch head-group partials, apply gating, transpose, add bo and
    the bv contribution (ctx += bv per head exactly, since probs sum to 1)."""
    bo = np.asarray(bo, np.float32)
    bv = np.asarray(bv, np.float32)
    Wo = np.asarray(Wo, np.float32)
    wobv = bv @ Wo  # [D]
    gpb = NCORES // B
    out = np.empty((B, S, D), np.float32)
    for b in range(B):
        acc = results[gpb * b]["outT"].astype(np.float32)
        for g in range(1, gpb):
            acc = acc + results[gpb * b + g]["outT"].astype(np.float32)
        out[b] = (acc.T + wobv[None, :]) * gating[b][:, None] + bo[None, :]
    return out


def kernel(hidden_states, Wq, bq, Wk, bk, Wv, bv, Wo, bo, gating_factor, gating_bias):
    in_maps = make_in_maps(
        hidden_states, Wq, bq, Wk, bk, Wv, bv, Wo, gating_factor, gating_bias
    )
    hs = np.asarray(hidden_states, np.float32)
    gf = np.float32(np.asarray(gating_factor, np.float32)[0])
    gb = np.float32(np.asarray(gating_bias, np.float32)[0])
    gating = 1.0 / (1.0 + np.exp(-(gf * hs.mean(axis=-1) + gb)))  # [B, S]
    res = run_gau(in_maps)
    return assemble_output(res.results, bo, bv, Wo, gating)


# revision 15
# speedup vs baseline: 1.0308x; 1.0308x over previous
"""GAU attention (gated attention unit) Trainium2 Bass kernel.

Reference computation (B=2, S=2048, D=1024, H=16, DH=64):
    q = (hs @ Wq + bq), k = (hs @ Wk + bk), v = (hs @ Wv + bv)   per-head [B,S,H,DH]
    scores = q k^T / sqrt(DH);  probs = softmax(scores, axis=k)
    gating = sigmoid(gf * mean_d(hs) + gb)          # [B, S] per (batch, query)
    ctx = (probs * gating) @ v;  out = ctx @ Wo + bo

Sharding: 8 cores = 2 batches x 4 head-groups (4 heads each).  Each core
computes out^T partial [D, S] for its (batch, head-group); host sums the 4
partials per batch, applies the per-(batch,query) gating scalar (it commutes
to the output), transposes, and adds bo.

Bias algebra (exact, done on host so the device never sees bk/bv):
  - bk: scores_q += q . bk is constant over the softmax (key) axis -> cancels.
  - bv: ctx = sum_j p_j (v_j + bv) = sum_j p_j v_j + bv (probs sum to 1 per
    head), so out += Wo^T bv, applied on the host as a gating outer product.

Per-core dataflow (all matmuls bf16 with fp32 PSUM accumulation):
  - hs^T [D,S] staged bf16 (host transposes + casts).
  - Q^T,K^T [256,S]: lhsT=W tiles (stationary), rhs=hs^T.  Layout: pair p of
    heads stacked on partitions (head A dh on 0:64, head B on 64:128).
  - K-proj/V-proj/Q-proj are emitted just-in-time inside the first q-chunk's
    attention loop so the exp (ACT) stream starts early.
  - scores^T [k,q] per (pr, kt): two row-packed (tile_position (0,0)/(64,0))
    K=64 matmuls -> exp -> E^T bf16.  Exps run on ACT (table exp, scale=ln2
    over log2-space scores) except a few per chunk offloaded to DVE via a
    Schraudolph bit-trick: i16 = round(y*128 + (127-C)*128) reinterpreted as
    bf16 gives 2^y to ~2%; softmax-consistent so it perturbs ctx ~0.1%.
  - softmax denom: E^T ktiles folded into 4 partial sums per pr (kt%4
    chains) -- 3 chains on DVE, 1 on GPSIMD -- then col-packed ones-matmuls
    broadcast both heads' denominators into one [128,GQ] PSUM tile
    (accumulating the 4 partials), one reciprocal, one multiply.
  - AV: col-packed (tile_position (0,0)/(0,64)) matmuls, V stationary,
    E^T streaming -> ctx^T accumulated over ktiles in a single PSUM bank per
    pr (disjoint partition ranges; skip_group_check).
  - O-proj lhsT=Wo, rhs=ctx^T (bf16, already 1/denom-scaled) -> out^T.
  - Prologue: critical DMA (wk/wq m-halves, hs chunk0 d-tiles) spread over 4
    DGE rings (sync/scalar/gpsimd/vector) in consumption order; PE warm-up
    matmuls bridge the DMA window so projections start at full clock.
  - Tail: the last chunk's denominator/O-proj pipeline is split into q-halves
    of 256 and the output stores fan out over 3 rings.
"""

import sys

for _p in ("/opt/trn_rl_repo", "/root/.axon_site/_ro/trn_rl_repo"):
    if _p not in sys.path:
        sys.path.append(_p)

from contextlib import ExitStack

import ml_dtypes
import numpy as np

import concourse.bass as bass
import concourse.mybir as mybir
import concourse.tile as tile
from concourse import bacc
from concourse.bass_utils import run_bass_kernel_spmd

BF16 = mybir.dt.bfloat16
F32 = mybir.dt.float32
I16 = mybir.dt.int16
AF = mybir.ActivationFunctionType
OP = mybir.AluOpType

B, S, D, H = 2, 2048, 1024, 16
DH = 64
LN2 = float(np.log(2.0))
LOG2E = float(np.log2(np.e))
HPC = 4  # heads per core
GD = HPC * DH  # 256 (head-group width)
NCORES = 8
NDT = D // 128  # 8 contraction tiles over D

# Schraudolph 2^y for bf16: i16 = round(y*128 + (127 - C)*128), bits = bf16.
# C = 0.0430 is the mean-error-minimizing shift; +0.5 centers a truncating
# float->int conversion (harmless if the hw rounds: then it's C-1/256).
SCHRA_SCALE = 128.0
SCHRA_BIAS = (127.0 - 0.0430) * 128.0 + 0.5
# (qc>0 only) exps offloaded from ACT to DVE: pr==1 and kt in this set
DVE_EXP_KT = (3, 7, 11)


def _build(ctx: ExitStack, tc: "tile.TileContext", io: dict, s: int):
    nc = tc.nc
    GQ = min(512, s)
    NQC = s // GQ  # q chunks
    NKT = s // 128  # k tiles

    hsT, wq, wk, wv, wo = io["hsT"], io["wq"], io["wk"], io["wv"], io["wo"]
    bq, outT = io["bq"], io["outT"]

    consts = ctx.enter_context(tc.tile_pool(name="consts", bufs=1))
    sb = ctx.enter_context(tc.tile_pool(name="sb", bufs=1))
    # et tiles must stay live for NPAR=4 k-tiles (first fold of each chain
    # consumes et(kt-4)), 2 tiles per kt -> 16 buffers
    etp = ctx.enter_context(tc.tile_pool(name="etp", bufs=16))
    ksp = ctx.enter_context(tc.tile_pool(name="ksp", bufs=2))
    outp = ctx.enter_context(tc.tile_pool(name="outp", bufs=8))
    # PSUM budget: 2x2 (scores, 2-bank tiles) + 2 (ctx, one bank per pr via
    # disjoint-partition accumulation groups) + 2 (vproj/denom/o-proj) = 8
    ps_mm = ctx.enter_context(tc.tile_pool(name="ps_mm", bufs=2, space="PSUM"))
    ps_ctx = ctx.enter_context(tc.tile_pool(name="ps_ctx", bufs=2, space="PSUM"))
    ps_o = ctx.enter_context(tc.tile_pool(name="ps_o", bufs=2, space="PSUM"))

    # ---- constants ----
    ones128 = consts.tile([128, 128], BF16, tag="ones128", name="ones128")
    nc.vector.memset(ones128[:], 1.0)

    bq_sb = consts.tile([128, 2], F32, tag="bq", name="bq")
    # explicit zero bias for Exp, written by DVE so the wait merges with the
    # DVE wait the exps already carry
    zbias = consts.tile([128, 1], F32, tag="zbias", name="zbias")
    nc.vector.memset(zbias[:], 0.0)
    # dummy exp as the very first ACT instruction: pulls the ~1.3us
    # ACT_TABLE_LOAD into the DMA-wait window
    warm = consts.tile([1, 1], F32, tag="warm", name="warm")
    nc.scalar.activation(warm[:], zbias[0:1, 0:1], AF.Exp, bias=zbias[0:1, 0:1], scale=1.0)

    # ---- weights + hs^T staged.  wk/wq live m-half-major [128,(m,d,128)] so
    # each m-half is one contiguous piece and kproj(m,0) starts on a 128KB
    # landing; hs^T as [128,(chunk,d,GQ)] with chunk0 sent per-d-tile. ----
    wk_all = consts.tile([128, NDT * GD], BF16, tag="wk", name="wk")
    wq_all = consts.tile([128, NDT * GD], BF16, tag="wq", name="wq")
    wv_all = consts.tile([128, NDT * GD], BF16, tag="wv", name="wv")
    CW = NDT * GQ  # 4096 columns per hs chunk block
    hsT_all = sb.tile([128, NDT * s], BF16, tag="hsT", name="hsT")

    def wslice(wall, m, d):  # [128,128] stationary tile for (m-half, d-tile)
        off = (m * NDT + d) * 128
        return wall[:, off : off + 128]

    wv_sb = [wv_all[:, d * GD : (d + 1) * GD] for d in range(NDT)]

    def hsq(d, qc):  # [128, GQ] tile of hs^T for (d-tile, q-chunk)
        off = qc * CW + d * GQ
        return hsT_all[:, off : off + GQ]

    def hsv(d, kt):  # [128, 128] tile of hs^T for (d-tile, k-tile)
        c, r = divmod(kt, 4)
        off = c * CW + d * GQ + r * 128
        return hsT_all[:, off : off + 128]

    # ---- DMA schedule: 3 DGE rings (sync/scalar/gpsimd), consumption order.
    # sync:   bq, wk-m0 (2 pieces), wq-m0, wk-m1, wq-m1, hs c1..c3, wo
    # scalar: hs-c0 odd d-tiles only (ring free again before the exp stream)
    # gpsimd: hs-c0 even d-tiles, wv
    HWM = NDT * 128  # 1024 cols per m-half of a weight tensor
    nc.sync.dma_start(bq_sb[:], bq.rearrange("(m p) -> p m", p=128))
    for dh in range(2):  # wk m0 in two 128KB pieces (d0-3, d4-7)
        cs = slice(dh * HWM // 2, (dh + 1) * HWM // 2)
        nc.sync.dma_start(wk_all[:, cs], wk[:, cs])
    for d in range(NDT):  # hs chunk0 per d-tile, alternating idle rings
        eng = nc.gpsimd if d % 2 == 0 else nc.scalar
        eng.dma_start(hsT_all[:, d * GQ : (d + 1) * GQ], hsT[:, d * GQ : (d + 1) * GQ])
    nc.sync.dma_start(wq_all[:, 0:HWM], wq[:, 0:HWM])
    nc.sync.dma_start(wk_all[:, HWM : 2 * HWM], wk[:, HWM : 2 * HWM])
    nc.sync.dma_start(wq_all[:, HWM : 2 * HWM], wq[:, HWM : 2 * HWM])
    nc.gpsimd.dma_start(wv_all[:], wv[:, :])
    for c in range(1, NQC):
        nc.sync.dma_start(hsT_all[:, c * CW : (c + 1) * CW], hsT[:, c * CW : (c + 1) * CW])
    wo_sb = [consts.tile([128, D], BF16, tag=f"wo{p}", name=f"wo{p}") for p in range(2)]
    nc.sync.dma_start(wo_sb[0][:], wo[0:128, :])
    nc.sync.dma_start(wo_sb[1][:], wo[128:256, :])

    qT_sb = [sb.tile([128, s], BF16, tag=f"qT{m}", name=f"qT{m}") for m in range(2)]
    kT_sb = [sb.tile([128, s], BF16, tag=f"kT{m}", name=f"kT{m}") for m in range(2)]
    v_sb = [sb.tile([128, GD], BF16, tag=f"v{st}", name=f"v{st}") for st in range(NKT)]

    # d-iteration order for the first projections: evens first (gpsimd ring
    # delivers hs-c0 evens progressively) then odds (scalar ring, parallel);
    # fp32 PSUM accumulation commutes so any order is exact
    D_ARRIVAL = (0, 2, 4, 6, 1, 3, 5, 7)

    def kproj(m, c):
        cc = slice(c * GQ, (c + 1) * GQ)
        p = ps_o.tile([128, GQ], F32, tag="po", name=f"kp{m}{c}")
        order = D_ARRIVAL if c == 0 else range(NDT)
        for i, d in enumerate(order):
            nc.tensor.matmul(
                p[:], lhsT=wslice(wk_all, m, d), rhs=hsq(d, c),
                start=(i == 0), stop=(i == NDT - 1),
            )
        nc.vector.tensor_copy(kT_sb[m][:, cc], p[:])

    def qproj(m, qc):
        cc = slice(qc * GQ, (qc + 1) * GQ)
        p = ps_o.tile([128, GQ], F32, tag="po", name=f"qp{m}{qc}")
        for d in range(NDT):
            nc.tensor.matmul(
                p[:], lhsT=wslice(wq_all, m, d), rhs=hsq(d, qc),
                start=(d == 0), stop=(d == NDT - 1),
            )
        nc.vector.tensor_scalar_add(qT_sb[m][:, cc], p[:], bq_sb[:, m : m + 1])

    qchains = {}

    def qproj_half(m, qc, half):
        if half == 0:
            qchains[(m, qc)] = ps_o.tile([128, GQ], F32, tag="po", name=f"qph{m}{qc}")
        p = qchains[(m, qc)]
        for d in range(half * 4, half * 4 + 4):
            nc.tensor.matmul(
                p[:], lhsT=wslice(wq_all, m, d), rhs=hsq(d, qc),
                start=(d == 0), stop=(d == NDT - 1),
            )
        if half == 1:
            cc = slice(qc * GQ, (qc + 1) * GQ)
            nc.vector.tensor_scalar_add(qT_sb[m][:, cc], p[:], bq_sb[:, m : m + 1])

    def vproj(kt):
        vp = ps_o.tile([128, GD], F32, tag="po", name=f"vp{kt}")
        for d in range(NDT):
            nc.tensor.matmul(
                vp[:], lhsT=hsv(d, kt), rhs=wv_sb[d][:],
                start=(d == 0), stop=(d == NDT - 1),
            )
        nc.vector.tensor_copy(v_sb[kt][:], vp[:])

    # ---- PE warm-up: ~4.3us of dependency-free matmuls so the HAM clock
    # gate opens before the real (DMA-gated) projections run ----
    warm_ps = ps_o.tile([128, 128], F32, tag="po", name="warm_ps")
    NWARM = 40
    for i in range(NWARM):
        nc.tensor.matmul(
            warm_ps[:], lhsT=ones128[:], rhs=ones128[:],
            start=(i == 0), stop=(i == NWARM - 1),
        )

    # ---- prologue: just the pr0 projections; pr1's come as slot-0 filler
    # so the first exp fires after only two projection chains ----
    kproj(0, 0)
    qproj(0, 0)

    def oproj_unit(mt, ctx_sc, cs, copy_eng, dma_eng):
        ms = slice(mt * 128, (mt + 1) * 128)
        o_ps = ps_o.tile([128, GQ], F32, tag="po", name="po")
        for pr in range(2):
            nc.tensor.matmul(
                o_ps[:], lhsT=wo_sb[pr][:, ms], rhs=ctx_sc[pr][:],
                start=(pr == 0), stop=(pr == 1),
            )
        ost = outp.tile([128, GQ], BF16, tag="ost", name="ost")
        copy_eng(ost[:], o_ps[:])
        # outT layout [128, (mt, s)]: every store is a contiguous 1KB line
        # per partition (strided [ms, cs] stores were 4x slower); the host
        # unshuffles
        dma_eng.dma_start(outT[:, mt * s + cs.start : mt * s + cs.stop], ost[:])

    # ---- per q-chunk attention, both head-pair groups interleaved per kt;
    # chunk qc-1's denominators and O-proj trickle into qc's slots as PE
    # filler AFTER qc's first scores, so the ACT exp stream never gaps at
    # chunk boundaries ----
    NPAR = 4  # denominator partial-sum chains per (qc, pr), keyed by kt%4
    oproj_pending: list = []
    denom_pending: list = []
    for qc in range(NQC):
        cs = slice(qc * GQ, (qc + 1) * GQ)
        ctx_ps = [ps_ctx.tile([128, GQ], F32, tag="ctx", name=f"ctx{pr}") for pr in range(2)]
        # per (pr, kt%4) partial exp-sums; chain 0 folds on GPSIMD (its last
        # fold lands at kt=12, hidden under kts 13-15), chains 1-3 on DVE
        ks = [[None] * NPAR, [None] * NPAR]
        ets = [[None] * NKT, [None] * NKT]
        for kt in range(NKT + 1):
            sps = [None, None]
            for pr in range(2):
                if kt < NKT:
                    ks_ = slice(kt * 128, (kt + 1) * 128)
                    sp = ps_mm.tile([128, 2 * GQ], F32, tag="smm", name="smm")
                    sps[pr] = sp
                    nc.tensor.matmul(
                        sp[:, 0:GQ], lhsT=kT_sb[pr][0:64, ks_], rhs=qT_sb[pr][0:64, cs],
                        tile_position=(0, 0), start=True, stop=True,
                    )
                    nc.tensor.matmul(
                        sp[:, GQ : 2 * GQ], lhsT=kT_sb[pr][64:128, ks_], rhs=qT_sb[pr][64:128, cs],
                        tile_position=(64, 0), start=True, stop=True,
                    )
                if kt == 0 and denom_pending:
                    # previous chunk's denominator for THIS pr, right after
                    # this pr's kt0 scores are in the queue
                    denom_pending.pop(0)()
                if pr == 0 and kt < NKT:
                    # just-in-time projections keep PE fed while ACT drains
                    # exps; emission slots chosen so a DMA-gated chain never
                    # sits in the in-order PE queue ahead of ready work
                    if qc == 0:
                        if kt == 0:
                            kproj(1, 0)
                            qproj(1, 0)
                        elif kt == 1:
                            vproj(0)
                            vproj(1)
                        if 1 <= kt < NKT - 1:
                            vproj(kt + 1)
                        if kt in (2, 6, 10):
                            kproj(0, kt // 4 + 1)
                        elif kt in (3, 7, 11):
                            kproj(1, (kt - 1) // 4 + 1)
                        elif kt == 12:
                            qproj(0, 1)
                        elif kt == 13:
                            qproj(1, 1)
                    else:
                        if qc < NQC - 1:
                            if kt in (5, 6):
                                qproj_half(0, qc + 1, kt - 5)
                            elif kt in (7, 8):
                                qproj_half(1, qc + 1, kt - 7)
                        if oproj_pending and kt >= 1 and (kt < 5 or kt > 8):
                            oproj_pending.pop(0)()
                if kt > 0:
                    pv = kt - 1
                    et = ets[pr][pv]
                    nc.tensor.matmul(
                        ctx_ps[pr][0:64, :], lhsT=v_sb[pv][:, pr * 128 : pr * 128 + 64],
                        rhs=et[:, 0:GQ], tile_position=(0, 0),
                        start=(pv == 0), stop=(pv == NKT - 1),
                        skip_group_check=True,
                    )
                    nc.tensor.matmul(
                        ctx_ps[pr][64:128, :], lhsT=v_sb[pv][:, pr * 128 + 64 : pr * 128 + 128],
                        rhs=et[:, GQ : 2 * GQ], tile_position=(0, 64),
                        start=(pv == 0), stop=(pv == NKT - 1),
                        skip_group_check=True,
                    )
            for pr in range(2):
                if kt < NKT:
                    et = etp.tile([128, 2 * GQ], BF16, tag="et", name="et")
                    ets[pr][kt] = et
                    if qc > 0 and pr == 1 and kt in DVE_EXP_KT:
                        # Schraudolph on DVE: scores arrive in log2 space
                        # (log2e/8 folded into Wq); one fused mul+add into an
                        # int16 view of the bf16 tile computes 2^y bitwise
                        nc.vector.tensor_scalar(
                            et[:].bitcast(I16), sps[pr][:],
                            SCHRA_SCALE, SCHRA_BIAS, op0=OP.mult, op1=OP.add,
                        )
                    else:
                        # exp(ln2*y) = 2^y on ACT
                        nc.scalar.activation(et[:], sps[pr][:], AF.Exp, bias=zbias[:, 0:1], scale=LN2)
                    par = kt % NPAR
                    if kt >= NPAR:
                        fold_eng = nc.gpsimd if par == 0 else nc.vector
                        if kt < 2 * NPAR:
                            # first fold is out-of-place (no seed copy needed)
                            kst = ksp.tile([128, 2 * GQ], BF16, tag=f"ks{pr}{par}", name=f"ks{pr}{par}")
                            ks[pr][par] = kst
                            fold_eng.tensor_tensor(kst[:], ets[pr][par][:], et[:], op=OP.add)
                        else:
                            kst = ks[pr][par]
                            fold_eng.tensor_tensor(kst[:], kst[:], et[:], op=OP.add)

        # softmax denominators: col-packed ones-matmul pairs broadcast both
        # heads into one bank, accumulating the 4 partial chains.  State is
        # bound via default args because deferred calls outlive the loop
        # iteration that created them.
        def denom_ctx(pr, ks_l=ks, ctx_l=ctx_ps, qcl=qc):
            db = ps_o.tile([128, GQ], F32, tag="po", name=f"db{pr}")
            for par in range(NPAR):
                for hh in range(2):
                    nc.tensor.matmul(
                        db[hh * 64 : (hh + 1) * 64, :],
                        lhsT=ones128[:, hh * 64 : (hh + 1) * 64],
                        rhs=ks_l[pr][par][:, hh * GQ : (hh + 1) * GQ],
                        tile_position=(0, hh * 64),
                        start=(par == 0), stop=(par == NPAR - 1),
                        skip_group_check=True,
                    )
            r = ksp.tile([128, GQ], F32, tag=f"r{pr}", name=f"r{pr}")
            nc.vector.reciprocal_approx_fast(r[:], db[:])
            sc = sb.tile([128, GQ], BF16, tag=f"ctxs{pr}_{qcl % 2}", name=f"ctxs{pr}_{qcl % 2}")
            nc.vector.tensor_tensor(sc[:], ctx_l[pr][:], r[:], op=OP.mult)
            return sc

        if qc < NQC - 1:
            # denominators + O-proj deferred into the next chunk's kt0/kt1+
            # slots so this chunk's exp stream hands off without a PE gap
            scs: list = [None, None]

            def make_denom(pr, dcf=denom_ctx, c=cs, scs_ref=scs):
                def run():
                    scs_ref[pr] = dcf(pr)
                    if pr == 1:
                        oproj_pending.extend(
                            (lambda mt=mt, sc2=scs_ref, cc=c: oproj_unit(
                                mt, sc2, cc, nc.vector.tensor_copy,
                                (nc.sync, nc.gpsimd)[mt % 2]))
                            for mt in range(D // 128)
                        )
                return run

            denom_pending = [make_denom(0), make_denom(1)]
        else:
            # tail: pr0's half of the O-projection overlaps pr1's softmax
            # drain; pr1's half is added on DVE, outputs on 3 DMA rings
            sc0 = denom_ctx(0)
            osts = []
            for mt in range(D // 128):
                o_ps = ps_o.tile([128, GQ], F32, tag="po", name="po")
                nc.tensor.matmul(o_ps[:], lhsT=wo_sb[0][:, mt * 128 : (mt + 1) * 128],
                                 rhs=sc0[:], start=True, stop=True)
                ost = outp.tile([128, GQ], BF16, tag="ost", name="ost")
                copy_eng = nc.vector.tensor_copy if mt % 2 == 0 else nc.scalar.copy
                copy_eng(ost[:], o_ps[:])
                osts.append(ost)
            sc1 = denom_ctx(1)
            for mt in range(D // 128):
                o_ps = ps_o.tile([128, GQ], F32, tag="po", name="po")
                nc.tensor.matmul(o_ps[:], lhsT=wo_sb[1][:, mt * 128 : (mt + 1) * 128],
                                 rhs=sc1[:], start=True, stop=True)
                nc.vector.tensor_tensor(osts[mt][:], osts[mt][:], o_ps[:], op=OP.add)
                dma_eng = (nc.sync, nc.scalar, nc.gpsimd)[mt % 3]
                dma_eng.dma_start(outT[:, mt * s + cs.start : mt * s + cs.stop], osts[mt][:])


def build_gau_nc(s: int = S, debug: bool = False):
    nc = bacc.Bacc("TRN2", target_bir_lowering=False, debug=debug, num_devices=NCORES)
    io = {
        "hsT": nc.dram_tensor("hsT", [128, (D // 128) * s], BF16, kind="ExternalInput").ap(),
        "wq": nc.dram_tensor("wq", [128, (D // 128) * GD], BF16, kind="ExternalInput").ap(),
        "wk": nc.dram_tensor("wk", [128, (D // 128) * GD], BF16, kind="ExternalInput").ap(),
        "wv": nc.dram_tensor("wv", [128, (D // 128) * GD], BF16, kind="ExternalInput").ap(),
        "wo": nc.dram_tensor("wo", [GD, D], BF16, kind="ExternalInput").ap(),
        "bq": nc.dram_tensor("bq", [GD], F32, kind="ExternalInput").ap(),
        "outT": nc.dram_tensor("outT", [128, (D // 128) * s], BF16, kind="ExternalOutput").ap(),
    }
    with tile.TileContext(nc) as tc:
        with ExitStack() as ctx:
            _build(ctx, tc, io, s)
    nc.compile()
    return nc


def make_in_maps(hidden_states, Wq, bq, Wk, bk, Wv, bv, Wo, gating_factor, gating_bias):
    """Shard full inputs into 8 per-core input maps (host-side prep)."""
    bf = ml_dtypes.bfloat16
    f32 = np.float32
    hs = np.asarray(hidden_states, f32)
    Wq, Wk, Wv, Wo = (np.asarray(a, f32) for a in (Wq, Wk, Wv, Wo))
    bq = np.asarray(bq, f32)

    # hs^T pre-shuffled to [128, (chunk, d, 512)] so the device DMA is a
    # plain contiguous 2D block; wq/wk to m-half-major [128, (m, d, 128)];
    # wv to [128, (d, GD)]
    def shuf_hs(a):  # a: [S, D]
        return np.ascontiguousarray(
            a.reshape(S // 512, 512, D // 128, 128).transpose(3, 0, 2, 1).reshape(128, -1)
        ).astype(bf)

    def shuf_w_m(w):  # w: [D, GD] -> [128, (m, d, 128)]
        return np.ascontiguousarray(
            w.reshape(D // 128, 128, 2, 128).transpose(1, 2, 0, 3).reshape(128, -1)
        ).astype(bf)

    def shuf_w(w):  # w: [D, GD] -> [128, (d, GD)]
        return np.ascontiguousarray(
            w.reshape(D // 128, 128, GD).transpose(1, 0, 2).reshape(128, -1)
        ).astype(bf)

    hsT_b = [shuf_hs(hs[b]) for b in range(B)]
    in_maps = []
    for c in range(NCORES):
        b, g = divmod(c, NCORES // B)
        cols = slice(g * GD, (g + 1) * GD)
        in_maps.append(
            {
                "hsT": hsT_b[b],
                "wq": shuf_w_m(Wq[:, cols] * np.float32(LOG2E / 8.0)),
                "wk": shuf_w_m(Wk[:, cols]),
                "wv": shuf_w(Wv[:, cols]),
                "wo": np.ascontiguousarray(Wo[cols, :]).astype(bf),
                "bq": np.ascontiguousarray(bq[cols] * np.float32(LOG2E / 8.0)),
            }
        )
    return in_maps


_NC_CACHE: dict = {}


def _get_nc(s: int = S):
    if s not in _NC_CACHE:
        _NC_CACHE[s] = build_gau_nc(s)
    return _NC_CACHE[s]


def run_gau(in_maps, **kwargs):
    nc = _get_nc(S)
    return run_bass_kernel_spmd(nc, in_maps, core_ids=list(range(NCORES)), **kwargs)


def assemble_output(results, bo, bv, Wo, gating):
    """Sum per-batch head-group partials, apply gating, transpose, add bo and
    the bv contribution (ctx += bv per head exactly, since probs sum to 1)."""
    bo = np.asarray(bo, np.float32)
    bv = np.asarray(bv, np.float32)
    Wo = np.asarray(Wo, np.float32)
    wobv = bv @ Wo  # [D]
    gpb = NCORES // B
    out = np.empty((B, S, D), np.float32)
    for b in range(B):
        acc = results[gpb * b]["outT"].astype(np.float32)
        for g in range(1, gpb):
            acc = acc + results[gpb * b + g]["outT"].astype(np.float32)
        # outT layout [128, (mt, s)] -> [D, S]
        accT = acc.reshape(128, D // 128, S).swapaxes(0, 1).reshape(D, S)
        out[b] = (accT.T + wobv[None, :]) * gating[b][:, None] + bo[None, :]
    return out


def kernel(hidden_states, Wq, bq, Wk, bk, Wv, bv, Wo, bo, gating_factor, gating_bias):
    in_maps = make_in_maps(
        hidden_states, Wq, bq, Wk, bk, Wv, bv, Wo, gating_factor, gating_bias
    )
    hs = np.asarray(hidden_states, np.float32)
    gf = np.float32(np.asarray(gating_factor, np.float32)[0])
    gb = np.float32(np.asarray(gating_bias, np.float32)[0])
    gating = 1.0 / (1.0 + np.exp(-(gf * hs.mean(axis=-1) + gb)))  # [B, S]
    res = run_gau(in_maps)
    return assemble_output(res.results, bo, bv, Wo, gating)


# revision 22
# speedup vs baseline: 1.0637x; 1.0319x over previous
"""GAU attention (gated attention unit) Trainium2 Bass kernel.

Reference computation (B=2, S=2048, D=1024, H=16, DH=64):
    q = (hs @ Wq + bq), k = (hs @ Wk + bk), v = (hs @ Wv + bv)   per-head [B,S,H,DH]
    scores = q k^T / sqrt(DH);  probs = softmax(scores, axis=k)
    gating = sigmoid(gf * mean_d(hs) + gb)          # [B, S] per (batch, query)
    ctx = (probs * gating) @ v;  out = ctx @ Wo + bo

Sharding: 8 cores = 2 batches x 4 head-groups (4 heads each).  Each core
computes out^T partial [D, S] for its (batch, head-group); host sums the 4
partials per batch, applies the per-(batch,query) gating scalar (it commutes
to the output), transposes, and adds bo.

Bias algebra (exact, done on host so the device never sees bk/bv):
  - bk: scores_q += q . bk is constant over the softmax (key) axis -> cancels.
  - bv: ctx = sum_j p_j (v_j + bv) = sum_j p_j v_j + bv (probs sum to 1 per
    head), so out += Wo^T bv, applied on the host as a gating outer product.

Per-core dataflow (all matmuls bf16 with fp32 PSUM accumulation):
  - hs^T [D,S] staged bf16 (host transposes + casts).
  - Q^T,K^T [256,S]: lhsT=W tiles (stationary), rhs=hs^T.  Layout: pair p of
    heads stacked on partitions (head A dh on 0:64, head B on 64:128).
  - K-proj/V-proj/Q-proj are emitted just-in-time inside the first q-chunk's
    attention loop so the exp (ACT) stream starts early.
  - scores^T [k,q] per (pr, kt): two row-packed (tile_position (0,0)/(64,0))
    K=64 matmuls -> exp -> E^T bf16.  Exps run on ACT (table exp, scale=ln2
    over log2-space scores) except a few per chunk offloaded to DVE via a
    Schraudolph bit-trick: i16 = round(y*128 + (127-C)*128) reinterpreted as
    bf16 gives 2^y to ~2%; softmax-consistent so it perturbs ctx ~0.1%.
  - softmax denom: E^T ktiles folded into 4 partial sums per pr (kt%4
    chains) -- 3 chains on DVE, 1 on GPSIMD -- then col-packed ones-matmuls
    broadcast both heads' denominators into one [128,GQ] PSUM tile
    (accumulating the 4 partials), one reciprocal, one multiply.
  - AV: col-packed (tile_position (0,0)/(0,64)) matmuls, V stationary,
    E^T streaming -> ctx^T accumulated over ktiles in a single PSUM bank per
    pr (disjoint partition ranges; skip_group_check).
  - O-proj lhsT=Wo, rhs=ctx^T (bf16, already 1/denom-scaled) -> out^T.
  - Prologue: critical DMA (wk/wq m-halves, hs chunk0 d-tiles) spread over 4
    DGE rings (sync/scalar/gpsimd/vector) in consumption order; PE warm-up
    matmuls bridge the DMA window so projections start at full clock.
  - Tail: the last chunk's denominator/O-proj pipeline is split into q-halves
    of 256 and the output stores fan out over 3 rings.
"""

import sys

for _p in ("/opt/trn_rl_repo", "/root/.axon_site/_ro/trn_rl_repo"):
    if _p not in sys.path:
        sys.path.append(_p)

from contextlib import ExitStack

import ml_dtypes
import numpy as np

import concourse.bass as bass
import concourse.mybir as mybir
import concourse.tile as tile
from concourse import bacc
from concourse.bass_utils import run_bass_kernel_spmd

BF16 = mybir.dt.bfloat16
F32 = mybir.dt.float32
I16 = mybir.dt.int16
AF = mybir.ActivationFunctionType
OP = mybir.AluOpType

B, S, D, H = 2, 2048, 1024, 16
DH = 64
LN2 = float(np.log(2.0))
LOG2E = float(np.log2(np.e))
HPC = 4  # heads per core
GD = HPC * DH  # 256 (head-group width)
NCORES = 8
NDT = D // 128  # 8 contraction tiles over D

# Schraudolph 2^y for bf16: i16 = round(y*128 + (127 - C)*128), bits = bf16.
# C = 0.0430 is the mean-error-minimizing shift; +0.5 centers a truncating
# float->int conversion (harmless if the hw rounds: then it's C-1/256).
SCHRA_SCALE = 128.0
SCHRA_BIAS = (127.0 - 0.0430) * 128.0 + 0.5
# (qc>0 only) exps offloaded from ACT to DVE: pr==1 and kt in this set
DVE_EXP_KT = (3, 7, 11)


def _build(ctx: ExitStack, tc: "tile.TileContext", io: dict, s: int):
    nc = tc.nc
    GQ = min(512, s)
    NQC = s // GQ  # q chunks
    NKT = s // 128  # k tiles

    hsT, wq, wk, wv, wo = io["hsT"], io["wq"], io["wk"], io["wv"], io["wo"]
    bq, outT = io["bq"], io["outT"]

    consts = ctx.enter_context(tc.tile_pool(name="consts", bufs=1))
    sb = ctx.enter_context(tc.tile_pool(name="sb", bufs=1))
    # et tiles must stay live for NPAR=4 k-tiles (first fold of each chain
    # consumes et(kt-4)), 2 tiles per kt -> 16 buffers
    etp = ctx.enter_context(tc.tile_pool(name="etp", bufs=16))
    ksp = ctx.enter_context(tc.tile_pool(name="ksp", bufs=2))
    outp = ctx.enter_context(tc.tile_pool(name="outp", bufs=2))
    # PSUM budget: 2x2 (scores, 2-bank tiles) + 2 (ctx, one bank per pr via
    # disjoint-partition accumulation groups) + 2 (vproj/denom/o-proj) = 8
    ps_mm = ctx.enter_context(tc.tile_pool(name="ps_mm", bufs=2, space="PSUM"))
    ps_ctx = ctx.enter_context(tc.tile_pool(name="ps_ctx", bufs=2, space="PSUM"))
    ps_o = ctx.enter_context(tc.tile_pool(name="ps_o", bufs=2, space="PSUM"))

    # ---- constants ----
    ones128 = consts.tile([128, 128], BF16, tag="ones128", name="ones128")
    nc.vector.memset(ones128[:], 1.0)

    bq_sb = consts.tile([128, 2], F32, tag="bq", name="bq")
    # explicit zero bias for Exp, written by DVE so the wait merges with the
    # DVE wait the exps already carry
    zbias = consts.tile([128, 1], F32, tag="zbias", name="zbias")
    nc.vector.memset(zbias[:], 0.0)
    # dummy exp as the very first ACT instruction: pulls the ~1.3us
    # ACT_TABLE_LOAD into the DMA-wait window
    warm = consts.tile([1, 1], F32, tag="warm", name="warm")
    nc.scalar.activation(warm[:], zbias[0:1, 0:1], AF.Exp, bias=zbias[0:1, 0:1], scale=1.0)

    # ---- weights + hs^T staged.  wk/wq live m-half-major [128,(m,d,128)] so
    # each m-half is one contiguous piece and kproj(m,0) starts on a 128KB
    # landing; hs^T as [128,(chunk,d,GQ)] with chunk0 sent per-d-tile. ----
    wk_all = consts.tile([128, NDT * GD], BF16, tag="wk", name="wk")
    wq_all = consts.tile([128, NDT * GD], BF16, tag="wq", name="wq")
    wv_all = consts.tile([128, NDT * GD], BF16, tag="wv", name="wv")
    CW = NDT * GQ  # 4096 columns per hs chunk block
    hsT_all = sb.tile([128, NDT * s], BF16, tag="hsT", name="hsT")

    def wslice(wall, m, d):  # [128,128] stationary tile for (m-half, d-tile)
        off = (m * NDT + d) * 128
        return wall[:, off : off + 128]

    wv_sb = [wv_all[:, d * GD : (d + 1) * GD] for d in range(NDT)]

    def hsq(d, qc):  # [128, GQ] tile of hs^T for (d-tile, q-chunk)
        off = qc * CW + d * GQ
        return hsT_all[:, off : off + GQ]

    def hsv(d, kt):  # [128, 128] tile of hs^T for (d-tile, k-tile)
        c, r = divmod(kt, 4)
        off = c * CW + d * GQ + r * 128
        return hsT_all[:, off : off + 128]

    # ---- DMA schedule: 3 DGE rings (sync/scalar/gpsimd), consumption order.
    # sync:   bq, wk-m0 (2 pieces), wq-m0, wk-m1, wq-m1, hs c1..c3, wo
    # scalar: hs-c0 odd d-tiles only (ring free again before the exp stream)
    # gpsimd: hs-c0 even d-tiles, wv
    HWM = NDT * 128  # 1024 cols per m-half of a weight tensor
    nc.sync.dma_start(bq_sb[:], bq.rearrange("(m p) -> p m", p=128))
    for dh in range(2):  # wk m0 in two 128KB pieces (d0-3, d4-7)
        cs = slice(dh * HWM // 2, (dh + 1) * HWM // 2)
        nc.sync.dma_start(wk_all[:, cs], wk[:, cs])
    # hs chunk0 in two 4-d-tile pieces: 4KB DRAM lines (smaller pieces were
    # line-size-throttled to ~50GB/s; 4KB lines run ~200GB/s)
    nc.gpsimd.dma_start(hsT_all[:, 0 : 4 * GQ], hsT[:, 0 : 4 * GQ])
    nc.scalar.dma_start(hsT_all[:, 4 * GQ : 8 * GQ], hsT[:, 4 * GQ : 8 * GQ])
    nc.sync.dma_start(wq_all[:, 0:HWM], wq[:, 0:HWM])
    nc.sync.dma_start(wk_all[:, HWM : 2 * HWM], wk[:, HWM : 2 * HWM])
    nc.sync.dma_start(wq_all[:, HWM : 2 * HWM], wq[:, HWM : 2 * HWM])
    nc.gpsimd.dma_start(wv_all[:], wv[:, :])
    for c in range(1, NQC):
        nc.sync.dma_start(hsT_all[:, c * CW : (c + 1) * CW], hsT[:, c * CW : (c + 1) * CW])
    wo_sb = [consts.tile([128, D], BF16, tag=f"wo{p}", name=f"wo{p}") for p in range(2)]
    nc.sync.dma_start(wo_sb[0][:], wo[0:128, :])
    nc.sync.dma_start(wo_sb[1][:], wo[128:256, :])

    qT_sb = [sb.tile([128, s], BF16, tag=f"qT{m}", name=f"qT{m}") for m in range(2)]
    kT_sb = [sb.tile([128, s], BF16, tag=f"kT{m}", name=f"kT{m}") for m in range(2)]
    v_sb = [sb.tile([128, GD], BF16, tag=f"v{st}", name=f"v{st}") for st in range(NKT)]

    def kproj(m, c):
        cc = slice(c * GQ, (c + 1) * GQ)
        p = ps_o.tile([128, GQ], F32, tag="po", name=f"kp{m}{c}")
        for d in range(NDT):
            nc.tensor.matmul(
                p[:], lhsT=wslice(wk_all, m, d), rhs=hsq(d, c),
                start=(d == 0), stop=(d == NDT - 1),
            )
        nc.vector.tensor_copy(kT_sb[m][:, cc], p[:])

    def qproj(m, qc):
        cc = slice(qc * GQ, (qc + 1) * GQ)
        p = ps_o.tile([128, GQ], F32, tag="po", name=f"qp{m}{qc}")
        for d in range(NDT):
            nc.tensor.matmul(
                p[:], lhsT=wslice(wq_all, m, d), rhs=hsq(d, qc),
                start=(d == 0), stop=(d == NDT - 1),
            )
        nc.vector.tensor_scalar_add(qT_sb[m][:, cc], p[:], bq_sb[:, m : m + 1])

    qchains = {}

    def qproj_half(m, qc, half):
        if half == 0:
            qchains[(m, qc)] = ps_o.tile([128, GQ], F32, tag="po", name=f"qph{m}{qc}")
        p = qchains[(m, qc)]
        for d in range(half * 4, half * 4 + 4):
            nc.tensor.matmul(
                p[:], lhsT=wslice(wq_all, m, d), rhs=hsq(d, qc),
                start=(d == 0), stop=(d == NDT - 1),
            )
        if half == 1:
            cc = slice(qc * GQ, (qc + 1) * GQ)
            nc.vector.tensor_scalar_add(qT_sb[m][:, cc], p[:], bq_sb[:, m : m + 1])

    def vproj(kt):
        vp = ps_o.tile([128, GD], F32, tag="po", name=f"vp{kt}")
        for d in range(NDT):
            nc.tensor.matmul(
                vp[:], lhsT=hsv(d, kt), rhs=wv_sb[d][:],
                start=(d == 0), stop=(d == NDT - 1),
            )
        nc.vector.tensor_copy(v_sb[kt][:], vp[:])

    # ---- PE warm-up: ~4.3us of dependency-free matmuls so the HAM clock
    # gate opens before the real (DMA-gated) projections run ----
    warm_ps = ps_o.tile([128, 128], F32, tag="po", name="warm_ps")
    NWARM = 40
    for i in range(NWARM):
        nc.tensor.matmul(
            warm_ps[:], lhsT=ones128[:], rhs=ones128[:],
            start=(i == 0), stop=(i == NWARM - 1),
        )

    # ---- prologue: just the pr0 projections; pr1's come as slot-0 filler
    # so the first exp fires after only two projection chains ----
    kproj(0, 0)
    qproj(0, 0)

    # O-proj results stage into one [128, 8*GQ] SBUF buffer per chunk, then
    # leave as two 4-mt-wide DMAs: 4KB DRAM lines run ~4x faster than the
    # 1KB-line per-mt stores.  outT layout [128, (qc, mt, GQ)].
    ostages: dict = {}

    def oproj_unit(mt, ctx_sc, qci, copy_eng):
        ms = slice(mt * 128, (mt + 1) * 128)
        o_ps = ps_o.tile([128, GQ], F32, tag="po", name="po")
        for pr in range(2):
            nc.tensor.matmul(
                o_ps[:], lhsT=wo_sb[pr][:, ms], rhs=ctx_sc[pr][:],
                start=(pr == 0), stop=(pr == 1),
            )
        if mt == 0:
            ostages[qci] = outp.tile([128, 8 * GQ], BF16, tag="ostage", name="ostage")
        stage = ostages[qci]
        copy_eng(stage[:, mt * GQ : (mt + 1) * GQ], o_ps[:])
        if mt in (3, 7):
            h = mt // 4
            eng = (nc.sync, nc.gpsimd)[h]
            off = (qci * 8 + h * 4) * GQ
            eng.dma_start(outT[:, off : off + 4 * GQ], stage[:, h * 4 * GQ : (h + 1) * 4 * GQ])

    # ---- per q-chunk attention, both head-pair groups interleaved per kt;
    # chunk qc-1's denominators and O-proj trickle into qc's slots as PE
    # filler AFTER qc's first scores, so the ACT exp stream never gaps at
    # chunk boundaries ----
    NPAR = 4  # denominator partial-sum chains per (qc, pr), keyed by kt%4
    oproj_pending: list = []
    denom_pending: list = []
    for qc in range(NQC):
        cs = slice(qc * GQ, (qc + 1) * GQ)
        ctx_ps = [ps_ctx.tile([128, GQ], F32, tag="ctx", name=f"ctx{pr}") for pr in range(2)]
        # per (pr, kt%4) partial exp-sums; chain 0 folds on GPSIMD (its last
        # fold lands at kt=12, hidden under kts 13-15), chains 1-3 on DVE
        ks = [[None] * NPAR, [None] * NPAR]
        ets = [[None] * NKT, [None] * NKT]
        for kt in range(NKT + 1):
            sps = [None, None]
            for pr in range(2):
                if kt < NKT:
                    ks_ = slice(kt * 128, (kt + 1) * 128)
                    sp = ps_mm.tile([128, 2 * GQ], F32, tag="smm", name="smm")
                    sps[pr] = sp
                    nc.tensor.matmul(
                        sp[:, 0:GQ], lhsT=kT_sb[pr][0:64, ks_], rhs=qT_sb[pr][0:64, cs],
                        tile_position=(0, 0), start=True, stop=True,
                    )
                    nc.tensor.matmul(
                        sp[:, GQ : 2 * GQ], lhsT=kT_sb[pr][64:128, ks_], rhs=qT_sb[pr][64:128, cs],
                        tile_position=(64, 0), start=True, stop=True,
                    )
                if kt == 0 and denom_pending:
                    # previous chunk's denominator for THIS pr, right after
                    # this pr's kt0 scores are in the queue
                    denom_pending.pop(0)()
                if pr == 0 and kt < NKT:
                    # just-in-time projections keep PE fed while ACT drains
                    # exps; emission slots chosen so a DMA-gated chain never
                    # sits in the in-order PE queue ahead of ready work
                    if qc == 0:
                        if kt == 0:
                            kproj(1, 0)
                            qproj(1, 0)
                        elif kt == 1:
                            vproj(0)
                            vproj(1)
                        if 1 <= kt < NKT - 1:
                            vproj(kt + 1)
                        if kt in (2, 6, 10):
                            kproj(0, kt // 4 + 1)
                        elif kt in (3, 7, 11):
                            kproj(1, (kt - 1) // 4 + 1)
                        elif kt == 12:
                            qproj(0, 1)
                        elif kt == 13:
                            qproj(1, 1)
                    else:
                        if qc < NQC - 1:
                            if kt in (5, 6):
                                qproj_half(0, qc + 1, kt - 5)
                            elif kt in (7, 8):
                                qproj_half(1, qc + 1, kt - 7)
                        if oproj_pending and kt >= 1 and (kt < 5 or kt > 8):
                            oproj_pending.pop(0)()
                if kt > 0:
                    pv = kt - 1
                    et = ets[pr][pv]
                    nc.tensor.matmul(
                        ctx_ps[pr][0:64, :], lhsT=v_sb[pv][:, pr * 128 : pr * 128 + 64],
                        rhs=et[:, 0:GQ], tile_position=(0, 0),
                        start=(pv == 0), stop=(pv == NKT - 1),
                        skip_group_check=True,
                    )
                    nc.tensor.matmul(
                        ctx_ps[pr][64:128, :], lhsT=v_sb[pv][:, pr * 128 + 64 : pr * 128 + 128],
                        rhs=et[:, GQ : 2 * GQ], tile_position=(0, 64),
                        start=(pv == 0), stop=(pv == NKT - 1),
                        skip_group_check=True,
                    )
            for pr in range(2):
                if kt < NKT:
                    et = etp.tile([128, 2 * GQ], BF16, tag="et", name="et")
                    ets[pr][kt] = et
                    if qc > 0 and pr == 1 and kt in DVE_EXP_KT:
                        # Schraudolph on DVE: scores arrive in log2 space
                        # (log2e/8 folded into Wq); one fused mul+add into an
                        # int16 view of the bf16 tile computes 2^y bitwise
                        nc.vector.tensor_scalar(
                            et[:].bitcast(I16), sps[pr][:],
                            SCHRA_SCALE, SCHRA_BIAS, op0=OP.mult, op1=OP.add,
                        )
                    else:
                        # exp(ln2*y) = 2^y on ACT
                        nc.scalar.activation(et[:], sps[pr][:], AF.Exp, bias=zbias[:, 0:1], scale=LN2)
                    par = kt % NPAR
                    if kt >= NPAR:
                        fold_eng = nc.gpsimd if par == 0 else nc.vector
                        if kt < 2 * NPAR:
                            # first fold is out-of-place (no seed copy needed)
                            kst = ksp.tile([128, 2 * GQ], BF16, tag=f"ks{pr}{par}", name=f"ks{pr}{par}")
                            ks[pr][par] = kst
                            fold_eng.tensor_tensor(kst[:], ets[pr][par][:], et[:], op=OP.add)
                        else:
                            kst = ks[pr][par]
                            fold_eng.tensor_tensor(kst[:], kst[:], et[:], op=OP.add)

        # softmax denominators: col-packed ones-matmul pairs broadcast both
        # heads into one bank, accumulating the 4 partial chains.  State is
        # bound via default args because deferred calls outlive the loop
        # iteration that created them.
        def denom_ctx(pr, ks_l=ks, ctx_l=ctx_ps, qcl=qc):
            db = ps_o.tile([128, GQ], F32, tag="po", name=f"db{pr}")
            for par in range(NPAR):
                for hh in range(2):
                    nc.tensor.matmul(
                        db[hh * 64 : (hh + 1) * 64, :],
                        lhsT=ones128[:, hh * 64 : (hh + 1) * 64],
                        rhs=ks_l[pr][par][:, hh * GQ : (hh + 1) * GQ],
                        tile_position=(0, hh * 64),
                        start=(par == 0), stop=(par == NPAR - 1),
                        skip_group_check=True,
                    )
            r = ksp.tile([128, GQ], F32, tag=f"r{pr}", name=f"r{pr}")
            nc.vector.reciprocal_approx_fast(r[:], db[:])
            sc = sb.tile([128, GQ], BF16, tag=f"ctxs{pr}_{qcl % 2}", name=f"ctxs{pr}_{qcl % 2}")
            nc.vector.tensor_tensor(sc[:], ctx_l[pr][:], r[:], op=OP.mult)
            return sc

        if qc < NQC - 1:
            # denominators + O-proj deferred into the next chunk's kt0/kt1+
            # slots so this chunk's exp stream hands off without a PE gap
            scs: list = [None, None]

            def make_denom(pr, dcf=denom_ctx, qci=qc, scs_ref=scs):
                def run():
                    scs_ref[pr] = dcf(pr)
                    if pr == 1:
                        oproj_pending.extend(
                            (lambda mt=mt, sc2=scs_ref: oproj_unit(
                                mt, sc2, qci,
                                nc.vector.tensor_copy if mt % 2 == 0 else nc.scalar.copy))
                            for mt in range(D // 128)
                        )
                return run

            denom_pending = [make_denom(0), make_denom(1)]
        else:
            # tail: both denominators immediately (all folds are done ~1us
            # after the last exp), then the 8 O-proj units with copies
            # alternating DVE/ACT and the staged 2-piece store
            ctx_sc = [denom_ctx(0), denom_ctx(1)]
            for mt in range(D // 128):
                oproj_unit(mt, ctx_sc, qc,
                           nc.vector.tensor_copy if mt % 2 == 0 else nc.scalar.copy)


def build_gau_nc(s: int = S, debug: bool = False):
    nc = bacc.Bacc("TRN2", target_bir_lowering=False, debug=debug, num_devices=NCORES)
    io = {
        "hsT": nc.dram_tensor("hsT", [128, (D // 128) * s], BF16, kind="ExternalInput").ap(),
        "wq": nc.dram_tensor("wq", [128, (D // 128) * GD], BF16, kind="ExternalInput").ap(),
        "wk": nc.dram_tensor("wk", [128, (D // 128) * GD], BF16, kind="ExternalInput").ap(),
        "wv": nc.dram_tensor("wv", [128, (D // 128) * GD], BF16, kind="ExternalInput").ap(),
        "wo": nc.dram_tensor("wo", [GD, D], BF16, kind="ExternalInput").ap(),
        "bq": nc.dram_tensor("bq", [GD], F32, kind="ExternalInput").ap(),
        "outT": nc.dram_tensor("outT", [128, (s // min(512, s)) * (D // 128) * min(512, s)], BF16, kind="ExternalOutput").ap(),
    }
    with tile.TileContext(nc) as tc:
        with ExitStack() as ctx:
            _build(ctx, tc, io, s)
    nc.compile()
    return nc


def make_in_maps(hidden_states, Wq, bq, Wk, bk, Wv, bv, Wo, gating_factor, gating_bias):
    """Shard full inputs into 8 per-core input maps (host-side prep)."""
    bf = ml_dtypes.bfloat16
    f32 = np.float32
    hs = np.asarray(hidden_states, f32)
    Wq, Wk, Wv, Wo = (np.asarray(a, f32) for a in (Wq, Wk, Wv, Wo))
    bq = np.asarray(bq, f32)

    # hs^T pre-shuffled to [128, (chunk, d, 512)] so the device DMA is a
    # plain contiguous 2D block; wq/wk to m-half-major [128, (m, d, 128)];
    # wv to [128, (d, GD)]
    def shuf_hs(a):  # a: [S, D]
        return np.ascontiguousarray(
            a.reshape(S // 512, 512, D // 128, 128).transpose(3, 0, 2, 1).reshape(128, -1)
        ).astype(bf)

    def shuf_w_m(w):  # w: [D, GD] -> [128, (m, d, 128)]
        return np.ascontiguousarray(
            w.reshape(D // 128, 128, 2, 128).transpose(1, 2, 0, 3).reshape(128, -1)
        ).astype(bf)

    def shuf_w(w):  # w: [D, GD] -> [128, (d, GD)]
        return np.ascontiguousarray(
            w.reshape(D // 128, 128, GD).transpose(1, 0, 2).reshape(128, -1)
        ).astype(bf)

    hsT_b = [shuf_hs(hs[b]) for b in range(B)]
    in_maps = []
    for c in range(NCORES):
        b, g = divmod(c, NCORES // B)
        cols = slice(g * GD, (g + 1) * GD)
        in_maps.append(
            {
                "hsT": hsT_b[b],
                "wq": shuf_w_m(Wq[:, cols] * np.float32(LOG2E / 8.0)),
                "wk": shuf_w_m(Wk[:, cols]),
                "wv": shuf_w(Wv[:, cols]),
                "wo": np.ascontiguousarray(Wo[cols, :]).astype(bf),
                "bq": np.ascontiguousarray(bq[cols] * np.float32(LOG2E / 8.0)),
            }
        )
    return in_maps


_NC_CACHE: dict = {}


def _get_nc(s: int = S):
    if s not in _NC_CACHE:
        _NC_CACHE[s] = build_gau_nc(s)
    return _NC_CACHE[s]


def run_gau(in_maps, **kwargs):
    nc = _get_nc(S)
    return run_bass_kernel_spmd(nc, in_maps, core_ids=list(range(NCORES)), **kwargs)


def assemble_output(results, bo, bv, Wo, gating):
    """Sum per-batch head-group partials, apply gating, transpose, add bo and
    the bv contribution (ctx += bv per head exactly, since probs sum to 1)."""
    bo = np.asarray(bo, np.float32)
    bv = np.asarray(bv, np.float32)
    Wo = np.asarray(Wo, np.float32)
    wobv = bv @ Wo  # [D]
    gpb = NCORES // B
    out = np.empty((B, S, D), np.float32)
    for b in range(B):
        acc = results[gpb * b]["outT"].astype(np.float32)
        for g in range(1, gpb):
            acc = acc + results[gpb * b + g]["outT"].astype(np.float32)
        # outT layout [128, (qc, mt, GQ)] -> [S, D]
        GQ = 512
        a4 = acc.reshape(128, S // GQ, D // 128, GQ)  # [p, qc, mt, q]
        o = a4.transpose(1, 3, 2, 0).reshape(S, D)  # [qc*q, mt*p]
        out[b] = (o + wobv[None, :]) * gating[b][:, None] + bo[None, :]
    return out


def kernel(hidden_states, Wq, bq, Wk, bk, Wv, bv, Wo, bo, gating_factor, gating_bias):
    in_maps = make_in_maps(
        hidden_states, Wq, bq, Wk, bk, Wv, bv, Wo, gating_factor, gating_bias
    )
    hs = np.asarray(hidden_states, np.float32)
    gf = np.float32(np.asarray(gating_factor, np.float32)[0])
    gb = np.float32(np.asarray(gating_bias, np.float32)[0])
    gating = 1.0 / (1.0 + np.exp(-(gf * hs.mean(axis=-1) + gb)))  # [B, S]
    res = run_gau(in_maps)
    return assemble_output(res.results, bo, bv, Wo, gating)
